# revision 20
# baseline (speedup 1.0000x reference)
"""Trainium2 Bass kernel for nn_DiffusionActionHead (B=8, S=2048, D=4096).

Strategy (8 NeuronCores), v2 rewrite for overlap:
  - Data-parallel over batch for llm_output; tensor-parallel weight reads
    (core i: head-slice i of wv/wo, hidden-slice i of mlp_w1/mlp_w2), tiny
    diffusion tail replicated.
  - MAP-head attention with q_len=1 collapsed algebraically (U = wk@q/sqrt
    folded on host, f8e4); scores run as fp8 DoubleRow matmuls (K=256/mm).
  - ONE fat ordered DMA stream on the sync HWDGE queue:
      llmT(f8) -> llm(f16) -> wv -> wo -> rp -> blk -> w1 -> w2
    so weight tiles stream exactly in consumption order and automatically
    chase the llm phase; a thin scalar HWDGE queue carries the latency-
    critical small DMAs (collective staging, xbar transposes).
  - All nat->T transposes via xbar DMA-transpose (tdma), not PE.
  - LN after the attention AllReduce is folded into mm1 algebraically:
      h1 = r*(a@W1g) - r*mu*csum(W1g) + b1'
    PSUM accumulates a@W1g (starts as soon as `a` arrives; LN stats run
    in parallel on VectorE), then a single K=2 matmul adds the rank-1
    corrections (-mu (x) csum + (1/r) (x) b1'), and the Gelu eviction
    applies the per-batch scale r.  Same trick for the 3 tail blocks.
  - Keep-alive: tiny matmuls gated on each weight-tile DMA prevent the PE
    HAM clock-gate from re-throttling during collective gaps.
  - 3 collectives: AllToAll(pooled f16), AllReduce(attn f16),
    AllReduce(z f32); rin_w folded through mm2 by linearity as before.
"""

import numpy as np
import sys

if "/opt/trn_rl_repo" not in sys.path:
    sys.path.insert(0, "/opt/trn_rl_repo")

import concourse.bass as bass
import concourse.tile as tile
from concourse import bacc, mybir
from concourse.masks import make_identity
from concourse.bass_utils import run_bass_kernel_spmd

F32 = mybir.dt.float32
F16 = mybir.dt.float16
I32 = mybir.dt.int32
F8 = mybir.dt.float8e4
AF = mybir.ActivationFunctionType
ALU = mybir.AluOpType
DR = mybir.MatmulPerfMode.DoubleRow

B, S, D = 8, 2048, 4096
H, AD, TD, HID, NBLK = 8, 7, 32, 256, 3
DH = D // H            # 512
NC = 8                 # cores
P = 128
SC = S // P            # 16 S-chunks
DC = D // P            # 32 D-chunks
HD2 = D // 2           # 2048
F1S = 4 * D // NC      # 2048 per-core hidden cols of mlp_w1
FC = F1S // P          # 16
HC = HID // P          # 2
TWO_PI = 2.0 * float(np.pi)


def _bcast(src_ap, nparts):
    ap = src_ap
    assert ap.shape[0] == 1, ap.shape
    return bass.AP(tensor=ap.tensor, offset=ap.offset,
                   ap=[[0, nparts]] + [list(x) for x in ap.ap[1:]])


def build_program():
    nc = bacc.Bacc("TRN2", target_bir_lowering=False, debug=False,
                   num_devices=NC)
    t = {}

    def din(name, shape, dtype=F32):
        t[name] = nc.dram_tensor(name, shape, dtype, kind="ExternalInput")

    din("llm", [S, D], F16); din("llmT", [D, S], F8)
    din("u_r", [P, DC, 16], F8)
    din("sc_inv", [H, 1])
    din("wv_r", [P, DC, DH], F16); din("bv16", [1, DH], F16)
    din("wo_r", [P, 4, D], F16); din("bo16", [1, D], F16)        # bo/8
    din("w1_r", [P, DC, F1S], F16)                               # g-folded
    din("fix1", [2, F1S], F16)        # row0: csum(w1g), row1: b1'
    din("w2_r", [P, FC, D], F16)      # b2 folded through rp into rb16
    din("four_w2", [TD, 1]); din("phase2", [TD, 1])
    din("timeT", [1, B]); din("naT", [AD, B], F16)
    din("cond_w1", [TD, 2 * TD], F16); din("cond_b1c", [2 * TD, 1])
    din("cond_w2", [2 * TD, TD], F16); din("cond_b2c", [TD, 1])
    din("rin_cond8", [TD, HID], F16)           # rin_w[cond rows] / 8
    din("rp_r", [P, DC, HID], F16)             # rin_w[pooled rows] p-major
    din("rin_na8", [AD, HID], F16)             # rin_w[na rows] / 8
    din("rb16", [1, HID], F16)                 # rin_b / 8
    din("bw1_r", [P, NBLK, HC, 4 * HID], F16)  # g-folded
    din("fixb", [2, NBLK, 4 * HID], F16)       # row0: csum, row1: b1'
    din("bw2_r", [P, NBLK, 4 * HID // P, HID], F16)
    din("blk_b2_16", [NBLK, HID], F16)
    din("out_w", [HID, AD], F16); din("out_bc", [1, AD])
    t["res"] = nc.dram_tensor("res", [B, AD], F32, kind="ExternalOutput")

    t["cc_pool_in"] = nc.dram_tensor("cc_pool_in", [H, D], F16)
    t["cc_pool_out"] = nc.dram_tensor("cc_pool_out", [B, D], F16)
    t["cc_attn_in"] = nc.dram_tensor("cc_attn_in", [B, D], F16)
    t["cc_attn_out"] = nc.dram_tensor("cc_attn_out", [B, D], F16,
                                      addr_space="Shared")
    t["cc_z_in"] = nc.dram_tensor("cc_z_in", [B, HID], F32)
    t["cc_z_out"] = nc.dram_tensor("cc_z_out", [B, HID], F32,
                                   addr_space="Shared")

    with tile.TileContext(nc) as tc:
        import contextlib
        with contextlib.ExitStack() as ctx:
            _build(nc, tc, t, ctx)
    nc.finalize()
    return nc


def _build(nc, tc, t, ctx):
    GROUPS = [list(range(NC))]

    singles = ctx.enter_context(tc.tile_pool(name="singles", bufs=1))
    ltp = ctx.enter_context(tc.tile_pool(name="ltp", bufs=2))    # llmT ring
    lnp = ctx.enter_context(tc.tile_pool(name="lnp", bufs=2))    # llm ring
    wrp = ctx.enter_context(tc.tile_pool(name="wrp", bufs=9))   # weight ring
    vsc = ctx.enter_context(tc.tile_pool(name="vsc", bufs=8))    # vector scratch
    psb = ctx.enter_context(tc.tile_pool(name="psb", bufs=2, space="PSUM"))

    # ---- small persistent tiles -------------------------------------------
    ident = singles.tile([P, P], F32)
    make_identity(nc, ident)
    ident16 = singles.tile([P, P], F16)
    nc.vector.tensor_copy(out=ident16[:], in_=ident[:])
    ones8 = singles.tile([1, 16], F16)
    nc.vector.memset(ones8[:], 1.0)
    sh1_i = singles.tile([P, 1], I32)
    nc.vector.memset(sh1_i[:], 1)
    magic_i = singles.tile([P, 1], I32)
    nc.vector.memset(magic_i[:], 0x5F3759DF)

    # 16-row nat staging tiles for xbar transposes (rows 8..15 stay zero)
    pg16 = singles.tile([16, F1S], F16)       # p_nat, later g16
    nc.vector.memset(pg16[:], 0.0)
    ps16 = singles.tile([16, D], F16)         # poolh16, later sum_pre
    nc.vector.memset(ps16[:], 0.0)
    a16 = singles.tile([16, D], F16)          # attn after AllReduce
    nc.vector.memset(a16[:], 0.0)
    ctx16 = singles.tile([16, DH], F16)
    nc.vector.memset(ctx16[:], 0.0)
    x16 = singles.tile([16, HID], F16)
    nc.vector.memset(x16[:], 0.0)
    hb16 = singles.tile([16, 4 * HID], F16)
    nc.vector.memset(hb16[:], 0.0)
    # nat tiles without transposes (8 rows)
    pooled_nat = singles.tile([B, D], F16)    # pooled, later attn_part

    # transposed layouts
    pT = singles.tile([P, SC, 16], F16)
    poolhT = singles.tile([P, DC, 16], F16)
    ctxT = singles.tile([P, DH // P, 16], F16)
    aT = singles.tile([P, DC, 16], F16)
    gT = singles.tile([P, FC, 16], F16)
    spT = singles.tile([P, DC, 16], F16)
    xT = singles.tile([P, HC, 16], F16)
    xsT = singles.tile([P, HC, 16], F16)
    hbT = singles.tile([P, 4 * HID // P, 16], F16)

    # ---- constants (gpsimd SWDGE queue; all tiny) -------------------------
    u_sb = singles.tile([P, DC, 16], F8)
    nc.sync.dma_start(out=u_sb[:], in_=t["u_r"][:])
    sci_sb = singles.tile([H, 1], F32)
    nc.gpsimd.dma_start(out=sci_sb[:], in_=t["sc_inv"][:])
    bv_sb = singles.tile([1, DH], F16)
    nc.gpsimd.dma_start(out=bv_sb[:], in_=t["bv16"][:])
    bo_sb = singles.tile([1, D], F16)
    nc.gpsimd.dma_start(out=bo_sb[:], in_=t["bo16"][:])
    fix1_sb = singles.tile([2, F1S], F16)
    nc.gpsimd.dma_start(out=fix1_sb[:], in_=t["fix1"][:])
    rb_sb = singles.tile([1, HID], F16)
    nc.gpsimd.dma_start(out=rb_sb[:], in_=t["rb16"][:])
    fixb_sb = singles.tile([2, NBLK, 4 * HID], F16)
    nc.gpsimd.dma_start(out=fixb_sb[:], in_=t["fixb"][:])
    bb2_sb = singles.tile([1, NBLK, HID], F16)
    nc.gpsimd.dma_start(out=bb2_sb[:], in_=t["blk_b2_16"][:].rearrange("n f -> (n f)")[None, :])
    rc_sb = singles.tile([TD, HID], F16)
    nc.gpsimd.dma_start(out=rc_sb[:], in_=t["rin_cond8"][:])
    rna_sb = singles.tile([AD, HID], F16)
    nc.gpsimd.dma_start(out=rna_sb[:], in_=t["rin_na8"][:])
    naT_sb = singles.tile([AD, B], F16)
    nc.gpsimd.dma_start(out=naT_sb[:], in_=t["naT"][:])
    ow_sb = singles.tile([P, HC, AD], F16)
    nc.gpsimd.dma_start(out=ow_sb[:],
                        in_=t["out_w"][:].rearrange("(c p) a -> p c a", p=P))
    ob_bc = singles.tile([B, AD], F32)
    nc.gpsimd.dma_start(out=ob_bc[:], in_=_bcast(t["out_bc"][:], B))
    fw_sb = singles.tile([TD, 1], F32)
    nc.gpsimd.dma_start(out=fw_sb[:], in_=t["four_w2"][:])
    ph_sb = singles.tile([TD, 1], F32)
    nc.gpsimd.dma_start(out=ph_sb[:], in_=t["phase2"][:])
    tb32 = singles.tile([TD, B], F32)
    nc.gpsimd.dma_start(out=tb32[:], in_=_bcast(t["timeT"][:], TD))
    cw1_sb = singles.tile([TD, 2 * TD], F16)
    nc.gpsimd.dma_start(out=cw1_sb[:], in_=t["cond_w1"][:])
    cb1_sb = singles.tile([2 * TD, 1], F32)
    nc.gpsimd.dma_start(out=cb1_sb[:], in_=t["cond_b1c"][:])
    cw2_sb = singles.tile([2 * TD, TD], F16)
    nc.gpsimd.dma_start(out=cw2_sb[:], in_=t["cond_w2"][:])
    cb2_sb = singles.tile([TD, 1], F32)
    nc.gpsimd.dma_start(out=cb2_sb[:], in_=t["cond_b2c"][:])
    bw1_sb = singles.tile([P, NBLK, HC, 4 * HID], F16)
    bw2_sb = singles.tile([P, NBLK, 4 * HID // P, HID], F16)

    # ---- helpers ----------------------------------------------------------
    def tdma(dst_T, src16):
        """(16, c*128) f16 sbuf -> (128, c, 16) sbuf via xbar DMA transpose.
        dst[p, c, j] = src[j, c*128+p]; rows 8..15 of src are zero."""
        nc.scalar.dma_start(out=dst_T[:], in_=src16, transpose=True)

    def bias_mm(ps, bias_row, n_total, stop=True):
        nch = (n_total + 511) // 512
        for n in range(nch):
            w = min(512, n_total - n * 512)
            nc.tensor.matmul(ps[:, n * 512:n * 512 + w], ones8[:, :B],
                             bias_row[:, n * 512:n * 512 + w],
                             start=False, stop=(stop and n == nch - 1))

    def dummy_mm(wt_slice):
        """Tiny keep-alive matmul gated on a weight tile's DMA."""
        d_ps = psb.tile([8, 512], F32, tag="ps", name=f"dps_{dummy_mm.n}")
        dummy_mm.n += 1
        nc.tensor.matmul(d_ps[:8, :64], wt_slice[:, :8], wt_slice[:, :64],
                         start=True, stop=True)
    dummy_mm.n = 0

    def ln_stats(x_nat, npart, n, uid, newton=2):
        """Return (negmu_irr_row [2,8] f16 tile, r [npart,1] f32).
        negmu_irr row0 = -mean, row1 = 1/r = sqrt(var+eps)."""
        nsub = max(1, n // 512)
        st = vsc.tile([npart, nsub, nc.vector.BN_STATS_DIM], F32, tag="v",
                      name=f"lnst_{uid}")
        xg = x_nat.rearrange("p (a b) -> p a b", a=nsub)
        for g in range(nsub):
            nc.vector.bn_stats(out=st[:, g, :], in_=xg[:, g, :])
        mv = vsc.tile([npart, nc.vector.BN_AGGR_DIM], F32, tag="v",
                      name=f"lnmv_{uid}")
        nc.vector.bn_aggr(out=mv[:], in_=st[:])
        ve = vsc.tile([npart, 4], F32, tag="v", name=f"lnve_{uid}")
        nc.vector.tensor_scalar_add(out=ve[:, 0:1], in0=mv[:, 1:2], scalar1=1e-5)
        yi = vsc.tile([npart, 1], I32, tag="v", name=f"lnyi_{uid}")
        nc.vector.tensor_tensor(out=yi[:], in0=ve[:, 0:1].bitcast(I32),
                                in1=sh1_i[:npart, :],
                                op=ALU.logical_shift_right)
        nc.vector.tensor_tensor(out=yi[:], in0=magic_i[:npart, :], in1=yi[:],
                                op=ALU.subtract)
        r = yi[:].bitcast(F32)
        tt = vsc.tile([npart, 1], F32, tag="v", name=f"lntt_{uid}")
        for _ in range(newton):
            nc.vector.tensor_mul(out=tt[:], in0=r, in1=r)
            nc.vector.tensor_mul(out=tt[:], in0=tt[:], in1=ve[:, 0:1])
            nc.vector.tensor_scalar(out=tt[:], in0=tt[:], scalar1=-0.5,
                                    scalar2=1.5, op0=ALU.mult, op1=ALU.add)
            nc.vector.tensor_mul(out=yi[:].bitcast(F32), in0=r, in1=tt[:])
        # pack [-mu, 1/r] as two f16 columns, PE-transpose to a [2, 8] row pair
        pk = vsc.tile([npart, 2], F16, tag="v", name=f"lnpk_{uid}")
        nc.vector.tensor_scalar_mul(out=pk[:, 0:1], in0=mv[:, 0:1], scalar1=-1.0)
        nc.vector.tensor_mul(out=ve[:, 1:2], in0=ve[:, 0:1], in1=r)
        nc.vector.tensor_copy(out=pk[:, 1:2], in_=ve[:, 1:2])
        tp = psb.tile([8, 512], F16, tag="ps", name=f"lntp_{uid}")
        nc.tensor.transpose(tp[:2, :npart], pk[:], ident16[:npart, :npart])
        row = vsc.tile([2, 16], F16, tag="v", name=f"lnrow_{uid}")
        nc.vector.tensor_copy(out=row[:, :npart], in_=tp[:2, :npart])
        return row, r

    # =======================================================================
    # PE warmup (~3us of cold matmuls opens the HAM clock gate)
    # =======================================================================
    ps_w = psb.tile([8, 2048], F32, tag="ps", name="ps_warm")
    for i in range(26):
        nc.tensor.matmul(ps_w[:, :P], ident16[:, :B], ident16[:],
                         start=(i == 0), stop=(i == 25))

    # =======================================================================
    # STEP 1: scoresT (8, 2048) = U.T @ llmT   fp8 DoubleRow (K=256 per mm)
    # =======================================================================
    ps_sc = psb.tile([8, 2048], F32, tag="ps", name="ps_sc")
    llmT_r = t["llmT"].rearrange("(a p) s -> p a s", p=P)
    for j in range(DC // 4):
        lt = ltp.tile([P, 4, S], F8, tag="lt", name=f"llmT_t{j}")
        nc.sync.dma_start(out=lt[:], in_=llmT_r[:, 4 * j:4 * j + 4, :])
        for kk in range(2):
            k2 = 2 * j + kk          # DoubleRow pair index (of DC//2)
            for n in range(S // 512):
                nc.tensor.matmul(
                    ps_sc[:, n * 512:(n + 1) * 512],
                    u_sb[:, 4 * j + 2 * kk:4 * j + 2 * kk + 2, :8],
                    lt[:, 2 * kk:2 * kk + 2, n * 512:(n + 1) * 512],
                    start=(k2 == 0), stop=(k2 == DC // 2 - 1),
                    perf_mode=DR)

    # =======================================================================
    # STEP 2: softmax over S (shift-invariance: max-subtraction skipped,
    # |scores| is small so exp() is well-conditioned)
    # =======================================================================
    den = singles.tile([H, 1], F32)
    nc.scalar.activation(out=pg16[:8, :], in_=ps_sc[:], func=AF.Exp,
                         scale=sci_sb[:], accum_out=den[:])
    nc.vector.reciprocal(out=den[:], in_=den[:])
    tdma(pT, pg16[:])

    # =======================================================================
    # STEP 3: pooled (8, 4096) = pT.T @ llm ; AllToAll (head <-> batch)
    # =======================================================================
    ps_pA = psb.tile([8, 2048], F32, tag="ps", name="ps_poolA")
    ps_pB = psb.tile([8, 2048], F32, tag="ps", name="ps_poolB")
    llm_r = t["llm"].rearrange("(a p) d -> p a d", p=P)
    for s in range(SC):
        lt = lnp.tile([P, 1, D], F16, tag="ln", name=f"llm_t{s}")
        nc.sync.dma_start(out=lt[:], in_=llm_r[:, s:s + 1, :])
        for n in range(4):
            nc.tensor.matmul(ps_pA[:, n * 512:(n + 1) * 512],
                             pT[:, s, :8],
                             lt[:, 0, n * 512:(n + 1) * 512],
                             start=(s == 0), stop=(s == SC - 1))
        for n in range(4):
            nc.tensor.matmul(ps_pB[:, n * 512:(n + 1) * 512],
                             pT[:, s, :8],
                             lt[:, 0, HD2 + n * 512:HD2 + (n + 1) * 512],
                             start=(s == 0), stop=(s == SC - 1))
    # evict halves on two engines in parallel (8-lane ops are slow)
    nc.scalar.activation(out=pooled_nat[:, :HD2], in_=ps_pA[:],
                         func=AF.Identity, scale=den[:])
    nc.vector.tensor_scalar_mul(out=pooled_nat[:, HD2:], in0=ps_pB[:],
                                scalar1=den[:])
    nc.scalar.dma_start(out=t["cc_pool_in"][:], in_=pooled_nat[:])
    nc.gpsimd.collective_compute(
        "AllToAll", ALU.bypass, replica_groups=GROUPS,
        ins=[t["cc_pool_in"][:].opt()], outs=[t["cc_pool_out"][:].opt()])

    # ---- weight stream on the fat sync queue (starts after llm tiles) ----
    wv_tiles = []
    for g in range(4):
        wt = wrp.tile([P, 8, DH], F16, tag="w", name=f"wv_g{g}")
        nc.sync.dma_start(out=wt[:], in_=t["wv_r"][:, 8 * g:8 * (g + 1), :])
        dummy_mm(wt[:, 0, :])
        wv_tiles.append(wt)
    wo_tiles = []
    for g in range(4):
        wt = wrp.tile([P, 1, D], F16, tag="w", name=f"wo_g{g}")
        nc.sync.dma_start(out=wt[:], in_=t["wo_r"][:, g:g + 1, :])
        dummy_mm(wt[:, 0, :])
        wo_tiles.append(wt)
    rp_sb = singles.tile([P, DC, HID], F16)
    nc.sync.dma_start(out=rp_sb[:], in_=t["rp_r"][:])
    dummy_mm(rp_sb[:, 0, :])
    nc.sync.dma_start(out=bw1_sb[:], in_=t["bw1_r"][:])
    dummy_mm(bw1_sb[:, 0, 0, :])
    nc.sync.dma_start(out=bw2_sb[:], in_=t["bw2_r"][:])
    dummy_mm(bw2_sb[:, 0, 0, :])

    # ---- cond path (independent; scheduled into the A2A gap) --------------
    fu = singles.tile([TD, B], F32)
    nc.vector.tensor_scalar_mul(out=fu[:], in0=tb32[:], scalar1=fw_sb[:])
    fi = singles.tile([TD, B], I32)
    nc.vector.tensor_copy(out=fi[:], in_=fu[:])
    fif = singles.tile([TD, B], F32)
    nc.vector.tensor_copy(out=fif[:], in_=fi[:])
    nc.vector.tensor_sub(out=fu[:], in0=fu[:], in1=fif[:])
    ffT = singles.tile([TD, B], F16)
    nc.scalar.activation(out=ffT[:], in_=fu[:], func=AF.Sin,
                         scale=TWO_PI, bias=ph_sb[:])
    ps_c1 = psb.tile([P, 8], F32, tag="ps", name="ps_c1")
    nc.tensor.matmul(ps_c1[:2 * TD, :B], cw1_sb[:], ffT[:], start=True,
                     stop=True)
    c1 = singles.tile([2 * TD, B], F16)
    nc.scalar.activation(out=c1[:], in_=ps_c1[:2 * TD, :B], func=AF.Silu,
                         bias=cb1_sb[:])
    ps_c2 = psb.tile([P, 8], F32, tag="ps", name="ps_c2")
    nc.tensor.matmul(ps_c2[:TD, :B], cw2_sb[:], c1[:], start=True, stop=True)
    condT = singles.tile([TD, B], F16)
    nc.scalar.activation(out=condT[:], in_=ps_c2[:TD, :B], func=AF.Identity,
                         bias=cb2_sb[:])

    # =======================================================================
    # STEP 4: A2A out -> poolhT ; ctx (8, 512) = poolh @ wv + bv
    # =======================================================================
    nc.scalar.dma_start(out=ps16[:8, :], in_=t["cc_pool_out"][:])
    tdma(poolhT, ps16[:])
    ps_cx = psb.tile([8, 2048], F32, tag="ps", name="ps_cx")
    for g in range(4):
        for j in range(8):
            k = 8 * g + j
            nc.tensor.matmul(ps_cx[:, :DH], poolhT[:, k, :8],
                             wv_tiles[g][:, j, :],
                             start=(k == 0), stop=False)
    bias_mm(ps_cx[:, :DH], bv_sb, DH)
    nc.scalar.activation(out=ctx16[:8, :], in_=ps_cx[:, :DH], func=AF.Identity)
    tdma(ctxT, ctx16[:])

    # =======================================================================
    # STEP 5: attn partial (8, 4096) = ctx @ wo + bo/8 ; AllReduce (f16)
    # =======================================================================
    ps_aA = psb.tile([8, 2048], F32, tag="ps", name="ps_attnA")
    ps_aB = psb.tile([8, 2048], F32, tag="ps", name="ps_attnB")
    for k in range(4):
        for n in range(4):
            nc.tensor.matmul(ps_aA[:, n * 512:(n + 1) * 512],
                             ctxT[:, k, :8],
                             wo_tiles[k][:, 0, n * 512:(n + 1) * 512],
                             start=(k == 0), stop=False)
        for n in range(4):
            nc.tensor.matmul(ps_aB[:, n * 512:(n + 1) * 512],
                             ctxT[:, k, :8],
                             wo_tiles[k][:, 0, HD2 + n * 512:HD2 + (n + 1) * 512],
                             start=(k == 0), stop=False)
    bias_mm(ps_aA, bo_sb[:, :HD2], HD2)
    bias_mm(ps_aB, bo_sb[:, HD2:], HD2)
    nc.scalar.activation(out=pooled_nat[:, :HD2], in_=ps_aA[:],
                         func=AF.Identity)
    nc.vector.tensor_copy(out=pooled_nat[:, HD2:], in_=ps_aB[:])
    nc.scalar.dma_start(out=t["cc_attn_in"][:], in_=pooled_nat[:])
    nc.gpsimd.collective_compute(
        "AllReduce", ALU.add, replica_groups=GROUPS,
        ins=[t["cc_attn_in"][:].opt()], outs=[t["cc_attn_out"][:].opt()])

    # ---- w1 stream (chases the queue; mm1 consumes in order) --------------
    w1_tiles = []
    for g in range(16):
        wt = wrp.tile([P, 2, F1S], F16, tag="w", name=f"w1_g{g}")
        nc.sync.dma_start(out=wt[:], in_=t["w1_r"][:, 2 * g:2 * g + 2, :])
        if g < 8:
            # later tiles' ring slots are freed by mm1 itself, which sits
            # after these dummies in PE FIFO order -> would deadlock
            dummy_mm(wt[:, 0, :])
        w1_tiles.append(wt)

    # =======================================================================
    # STEP 6: a = AllReduce(attn); mm1 with LN folded algebraically:
    #   h1 = r*(a@W1g) - r*mu*csum + b1'  -> g = Gelu(r * PSUM)
    # =======================================================================
    nc.scalar.dma_start(out=a16[:8, :], in_=t["cc_attn_out"][:])
    tdma(aT, a16[:])
    row1, r1 = ln_stats(a16[:8, :], 8, D, "ln0")

    ps_h1 = psb.tile([8, 2048], F32, tag="ps", name="ps_h1")
    for g in range(16):
        for kk in range(2):
            k = 2 * g + kk
            for n in range(4):
                nc.tensor.matmul(ps_h1[:, n * 512:(n + 1) * 512],
                                 aT[:, k, :8],
                                 w1_tiles[g][:, kk, n * 512:(n + 1) * 512],
                                 start=(k == 0), stop=False)
    for n in range(4):   # rank-1 fixups: (-mu) x csum + (1/r) x b1'
        nc.tensor.matmul(ps_h1[:, n * 512:(n + 1) * 512], row1[:, :8],
                         fix1_sb[:, n * 512:(n + 1) * 512],
                         start=False, stop=(n == 3))
    nc.scalar.activation(out=pg16[:8, :], in_=ps_h1[:], func=AF.Gelu,
                         scale=r1)
    tdma(gT, pg16[:])

    # ---- w2 stream --------------------------------------------------------
    w2_tiles = []
    for g in range(16):
        wt = wrp.tile([P, 1, D], F16, tag="w", name=f"w2_g{g}")
        nc.sync.dma_start(out=wt[:], in_=t["w2_r"][:, g:g + 1, :])
        if g < 8:
            dummy_mm(wt[:, 0, :])
        w2_tiles.append(wt)

    # =======================================================================
    # STEP 7: h2 partial = g@w2 + b2/8 ; sum_pre = attn/8 + h2 ; z partial
    # =======================================================================
    nc.vector.tensor_scalar_mul(out=ps16[:8, :], in0=a16[:8, :], scalar1=0.125)
    for half in range(2):
        ps_h2 = psb.tile([8, 2048], F32, tag="ps", name=f"ps_h2_{half}")
        for g in range(16):
            k = g
            for n in range(4):
                nc.tensor.matmul(
                    ps_h2[:, n * 512:(n + 1) * 512], gT[:, k, :8],
                    w2_tiles[g][:, 0, half * HD2 + n * 512:half * HD2 + (n + 1) * 512],
                    start=(k == 0), stop=(k == 15 and n == 3))
        nc.vector.tensor_add(out=ps16[:8, half * HD2:(half + 1) * HD2],
                             in0=ps16[:8, half * HD2:(half + 1) * HD2],
                             in1=ps_h2[:])
    tdma(spT, ps16[:])

    ps_z = psb.tile([8, 2048], F32, tag="ps", name="ps_z")
    for k in range(DC):
        nc.tensor.matmul(ps_z[:, :HID], spT[:, k, :8], rp_sb[:, k, :],
                         start=(k == 0), stop=False)
    nc.tensor.matmul(ps_z[:, :HID], condT[:], rc_sb[:], start=False, stop=False)
    nc.tensor.matmul(ps_z[:, :HID], naT_sb[:], rna_sb[:], start=False, stop=False)
    bias_mm(ps_z[:, :HID], rb_sb, HID)
    z_nat = singles.tile([B, HID], F32)
    nc.vector.tensor_copy(out=z_nat[:], in_=ps_z[:, :HID])
    nc.scalar.dma_start(out=t["cc_z_in"][:], in_=z_nat[:])
    nc.gpsimd.collective_compute(
        "AllReduce", ALU.add, replica_groups=GROUPS,
        ins=[t["cc_z_in"][:].opt()], outs=[t["cc_z_out"][:].opt()])

    # =======================================================================
    # STEP 8: diffusion tail (replicated; LN folded via the same fixup)
    # =======================================================================
    x_nat = singles.tile([B, HID], F32)
    nc.scalar.dma_start(out=x_nat[:], in_=t["cc_z_out"][:])
    nc.vector.tensor_copy(out=x16[:8, :], in_=x_nat[:])
    tdma(xT, x16[:])

    for i in range(NBLK):
        rowb, rb_ = ln_stats(x_nat[:], 8, HID, f"lnb{i}", newton=2)
        ps_bh = psb.tile([8, 2048], F32, tag="ps", name=f"ps_bh_{i}")
        for k in range(HC):
            for n in range(2):
                nc.tensor.matmul(ps_bh[:, n * 512:(n + 1) * 512],
                                 xT[:, k, :8],
                                 bw1_sb[:, i, k, n * 512:(n + 1) * 512],
                                 start=(k == 0), stop=False)
        for n in range(2):
            nc.tensor.matmul(ps_bh[:, n * 512:(n + 1) * 512], rowb[:, :8],
                             fixb_sb[:, i, n * 512:(n + 1) * 512],
                             start=False, stop=(n == 1))
        nc.scalar.activation(out=hb16[:8, :], in_=ps_bh[:, :4 * HID],
                             func=AF.Silu, scale=rb_)
        tdma(hbT, hb16[:])

        ps_bo = psb.tile([8, 2048], F32, tag="ps", name=f"ps_bo_{i}")
        for k in range(4 * HID // P):
            nc.tensor.matmul(ps_bo[:, :HID], hbT[:, k, :8],
                             bw2_sb[:, i, k, :],
                             start=(k == 0), stop=False)
        bias_mm(ps_bo[:, :HID], bb2_sb[:, i, :], HID)
        nc.vector.tensor_add(out=x_nat[:], in0=x_nat[:], in1=ps_bo[:, :HID])
        if i < NBLK - 1:
            nc.vector.tensor_copy(out=x16[:8, :], in_=x_nat[:])
            tdma(xT, x16[:])

    # ---- final: res (8, 7) = swish(x) @ out_w + out_b
    nc.scalar.activation(out=x16[:8, :], in_=x_nat[:], func=AF.Silu)
    tdma(xsT, x16[:])
    ps_o = psb.tile([8, 2048], F32, tag="ps", name="ps_o")
    for k in range(HC):
        nc.tensor.matmul(ps_o[:8, :AD], xsT[:, k, :8], ow_sb[:, k, :],
                         start=(k == 0), stop=(k == HC - 1))
    out_sb = singles.tile([B, AD], F32)
    nc.vector.tensor_add(out=out_sb[:], in0=ps_o[:8, :AD], in1=ob_bc[:])
    nc.scalar.dma_start(out=t["res"][:], in_=out_sb[:])


_CACHED_NC = None


def _get_nc():
    global _CACHED_NC
    if _CACHED_NC is None:
        _CACHED_NC = build_program()
    return _CACHED_NC


def _prep_in_maps(inputs):
    f32 = np.float32
    f16 = np.float16
    llm_full = np.ascontiguousarray(np.asarray(inputs["llm_output"], dtype=f32))
    wq = np.asarray(inputs["wq"], f32); wk = np.asarray(inputs["wk"], f32)
    wv = np.asarray(inputs["wv"], f32); wo = np.asarray(inputs["wo"], f32)
    bq = np.asarray(inputs["bq"], f32); bv = np.asarray(inputs["bv"], f32)
    bo = np.asarray(inputs["bo"], f32)
    w1 = np.asarray(inputs["mlp_w1"], f32); b1 = np.asarray(inputs["mlp_b1"], f32)
    w2 = np.asarray(inputs["mlp_w2"], f32); b2 = np.asarray(inputs["mlp_b2"], f32)
    ln_g = np.asarray(inputs["ln_g"], f32); ln_b = np.asarray(inputs["ln_b"], f32)
    rin_w = np.asarray(inputs["rin_w"], f32)
    probe = np.asarray(inputs["probe"], f32).reshape(D)

    q = probe @ wq + bq
    U = np.empty((D, H), f32)
    for h in range(H):
        hs = slice(h * DH, (h + 1) * DH)
        U[:, h] = wk[:, hs] @ q[hs]
    U *= 1.0 / np.sqrt(DH)
    f8 = mybir.dt.np(F8)
    u_scale = float(2.0 ** np.floor(np.log2(64.0 / max(np.abs(U).max(), 1e-30))))
    u_pad = np.zeros((DC, P, 16), f32)
    u_pad[:, :, :H] = (U * u_scale).reshape(DC, P, H)

    # LN affine fold: LN(x)*g+b @ W == LN(x) @ (g*W) + b@W
    w1g = w1 * ln_g[:, None]
    b1_fold = b1 + ln_b @ w1                  # (4*D,)

    blk_g = np.asarray(inputs["blk_ln_g"], f32)
    blk_b = np.asarray(inputs["blk_ln_b"], f32)
    blk_w1 = np.asarray(inputs["blk_w1"], f32)
    blk_w2 = np.asarray(inputs["blk_w2"], f32)
    blk_b1 = np.asarray(inputs["blk_b1"], f32)
    bw1g = blk_w1 * blk_g[:, :, None]
    bb1_fold = blk_b1 + np.einsum("nh,nhf->nf", blk_b, blk_w1)
    fixb = np.stack([bw1g.sum(axis=1), bb1_fold])     # (2, NBLK, 4*HID)

    shared = {
        "u_r": np.ascontiguousarray(u_pad.transpose(1, 0, 2)).astype(f8),
        "sc_inv": np.full((H, 1), 1.0 / u_scale, f32),
        "bo16": (bo / NC).astype(f16).reshape(1, D),
        "four_w2": np.concatenate(
            [np.asarray(inputs["four_w"], f32).reshape(TD // 2, 1)] * 2),
        "phase2": np.concatenate(
            [np.full((TD // 2, 1), np.pi / 2, f32),
             np.zeros((TD // 2, 1), f32)]),
        "timeT": np.ascontiguousarray(np.asarray(inputs["time"], f32).T),
        "naT": np.ascontiguousarray(
            np.asarray(inputs["noisy_actions"], f32).T).astype(f16),
        "cond_w1": np.asarray(inputs["cond_w1"], f32).astype(f16),
        "cond_b1c": np.asarray(inputs["cond_b1"], f32).reshape(-1, 1),
        "cond_w2": np.asarray(inputs["cond_w2"], f32).astype(f16),
        "cond_b2c": np.asarray(inputs["cond_b2"], f32).reshape(-1, 1),
        "rin_cond8": (np.ascontiguousarray(rin_w[0:TD]) / NC).astype(f16),
        "rp_r": np.ascontiguousarray(
            rin_w[TD:TD + D].reshape(DC, P, HID).transpose(1, 0, 2)
        ).astype(f16),
        "rin_na8": (np.ascontiguousarray(rin_w[TD + D:]) / NC).astype(f16),
        "rb16": ((np.asarray(inputs["rin_b"], f32) + b2 @ rin_w[TD:TD + D])
                 / NC).astype(f16).reshape(1, HID),
        "bw1_r": np.ascontiguousarray(
            bw1g.reshape(NBLK, HC, P, 4 * HID).transpose(2, 0, 1, 3)
        ).astype(f16),
        "fixb": fixb.astype(f16),
        "bw2_r": np.ascontiguousarray(
            blk_w2.reshape(NBLK, 4 * HID // P, P, HID).transpose(2, 0, 1, 3)
        ).astype(f16),
        "blk_b2_16": np.asarray(inputs["blk_b2"], f32).astype(f16),
        "out_w": np.asarray(inputs["out_w"], f32).astype(f16),
        "out_bc": np.asarray(inputs["out_b"], f32).reshape(1, AD),
    }

    in_maps = []
    for i in range(NC):
        hb = slice(i * DH, (i + 1) * DH)
        fb = slice(i * F1S, (i + 1) * F1S)
        m = dict(shared)
        m["llm"] = llm_full[i].astype(f16)
        m["llmT"] = np.ascontiguousarray(llm_full[i].T).astype(f8)
        m["wv_r"] = np.ascontiguousarray(
            wv[:, hb].reshape(DC, P, DH).transpose(1, 0, 2)).astype(f16)
        m["bv16"] = np.ascontiguousarray(bv[hb]).astype(f16).reshape(1, DH)
        m["wo_r"] = np.ascontiguousarray(
            wo[hb, :].reshape(4, P, D).transpose(1, 0, 2)).astype(f16)
        w1s = w1g[:, fb]
        m["w1_r"] = np.ascontiguousarray(
            w1s.reshape(DC, P, F1S).transpose(1, 0, 2)).astype(f16)
        m["fix1"] = np.stack([w1s.sum(axis=0), b1_fold[fb]]).astype(f16)
        m["w2_r"] = np.ascontiguousarray(
            w2[fb, :].reshape(FC, P, D).transpose(1, 0, 2)).astype(f16)
        in_maps.append(m)
    return in_maps


def kernel(**inputs):
    nc = _get_nc()
    in_maps = _prep_in_maps(inputs)
    r = run_bass_kernel_spmd(nc, in_maps, core_ids=list(range(NC)))
    return np.ascontiguousarray(r.results[0]["res"]).astype(np.float32)


def run_traced(**inputs):
    nc = _get_nc()
    in_maps = _prep_in_maps(inputs)
    r = run_bass_kernel_spmd(nc, in_maps, core_ids=list(range(NC)), trace=True)
    return np.ascontiguousarray(r.results[0]["res"]).astype(np.float32), r


# revision 25
# speedup vs baseline: 1.2323x; 1.2323x over previous
"""Trainium2 Bass kernel for nn_DiffusionActionHead (B=8, S=2048, D=4096).

Strategy (8 NeuronCores), v3:
  - Data-parallel over batch for llm_output; tensor-parallel weight reads
    (core i: head-slice i of wv/wo, hidden-slice i of mlp_w1), tiny
    diffusion tail replicated.
  - MAP-head attention with q_len=1 collapsed algebraically (U = wk@q/sqrt
    folded on host, f8e4); scores run as fp8 DoubleRow matmuls (K=256/mm).
  - KEY FOLD: the MLP's second matmul output h2 = gelu(h1)@w2 feeds ONLY
    the z = pooled_out@rin projection, so w2 folds on the host:
    W2Q = w2_slice @ rin_w[pooled]  (2048x256, 1MB) -- the entire 16MB w2
    stream and mm2 phase disappear.  z = a@(rp/8) + g@W2Q + cond + na.
  - ONE fat ordered DMA stream on the sync HWDGE queue with a unified
    deep ring (2MB tiles):  llmT(f8) -> llm(f16) -> wv -> wo -> w1.
    The scalar HWDGE queue carries only latency-critical small DMAs
    (collective staging, xbar transposes) in dependency order.
  - All nat->T transposes via xbar DMA-transpose (tdma), not PE.
  - LN after the attention AllReduce is folded into mm1 algebraically:
      h1 = r*(a@W1g) - r*mu*csum(W1g) + b1'
    PSUM accumulates a@W1g immediately; LN stats run in parallel on
    VectorE; a single K=2 matmul adds the rank-1 corrections and the Gelu
    eviction applies the per-batch scale r.  Same trick in the 3 tail
    blocks (kills the LN -> matmul serialization).
  - Keep-alive: tiny matmuls gated on stream-tile DMAs prevent the PE HAM
    clock-gate from re-throttling during collective gaps (only where the
    gating cannot deadlock the PE FIFO).
  - 3 collectives: AllToAll(pooled f16), AllReduce(attn f16) -- needed
    only for LN stats + the a@rp term, AllReduce(z f32).
"""

import numpy as np
import sys

if "/opt/trn_rl_repo" not in sys.path:
    sys.path.insert(0, "/opt/trn_rl_repo")

import concourse.bass as bass
import concourse.tile as tile
from concourse import bacc, mybir
from concourse.masks import make_identity
from concourse.bass_utils import run_bass_kernel_spmd

F32 = mybir.dt.float32
F16 = mybir.dt.float16
I32 = mybir.dt.int32
F8 = mybir.dt.float8e4
AF = mybir.ActivationFunctionType
ALU = mybir.AluOpType
DR = mybir.MatmulPerfMode.DoubleRow

B, S, D = 8, 2048, 4096
H, AD, TD, HID, NBLK = 8, 7, 32, 256, 3
DH = D // H            # 512
NC = 8                 # cores
P = 128
SC = S // P            # 16 S-chunks
DC = D // P            # 32 D-chunks
HD2 = D // 2           # 2048
F1S = 4 * D // NC      # 2048 per-core hidden cols of mlp_w1
FC = F1S // P          # 16
HC = HID // P          # 2
TWO_PI = 2.0 * float(np.pi)


def _bcast(src_ap, nparts):
    ap = src_ap
    assert ap.shape[0] == 1, ap.shape
    return bass.AP(tensor=ap.tensor, offset=ap.offset,
                   ap=[[0, nparts]] + [list(x) for x in ap.ap[1:]])


def build_program():
    nc = bacc.Bacc("TRN2", target_bir_lowering=False, debug=False,
                   num_devices=NC)
    t = {}

    def din(name, shape, dtype=F32):
        t[name] = nc.dram_tensor(name, shape, dtype, kind="ExternalInput")

    din("llm", [S, D], F16); din("llmT", [D, S], F8)
    din("u_r", [P, DC, 16], F8)
    din("sc_inv", [H, 1])
    din("wv_r", [P, DC, DH], F16); din("bv16", [1, DH], F16)
    din("wo_r", [P, 4, D], F16); din("bo16", [1, D], F16)        # bo/8
    din("w1_r", [P, DC, F1S], F16)                               # g-folded
    din("fix1", [2, F1S], F16)        # row0: csum(w1g), row1: b1'
    din("w2q_r", [P, FC, HID], F16)   # w2 @ rin_w[pooled] fold (per core)
    din("four_w2", [TD, 1]); din("phase2", [TD, 1])
    din("timeT", [1, B]); din("naT", [AD, B], F16)
    din("cond_w1", [TD, 2 * TD], F16); din("cond_b1c", [2 * TD, 1])
    din("cond_w2", [2 * TD, TD], F16); din("cond_b2c", [TD, 1])
    din("rin_cond8", [TD, HID], F16)           # rin_w[cond rows] / 8
    din("rp8_r", [P, DC, HID], F16)            # rin_w[pooled rows] / 8
    din("rin_na8", [AD, HID], F16)             # rin_w[na rows] / 8
    din("rb16", [1, HID], F16)                 # (rin_b + b2@rp) / 8
    din("bw1_r", [P, NBLK, HC, 4 * HID], F16)  # g-folded
    din("fixb", [2, NBLK, 4 * HID], F16)       # row0: csum, row1: b1'
    din("bw2_r", [P, NBLK, 4 * HID // P, HID], F16)
    din("blk_b2_16", [NBLK, HID], F16)
    din("out_w", [HID, AD], F16); din("out_bc", [1, AD])
    t["res"] = nc.dram_tensor("res", [B, AD], F32, kind="ExternalOutput")

    t["cc_pool_in"] = nc.dram_tensor("cc_pool_in", [H, D], F16)
    t["cc_pool_out"] = nc.dram_tensor("cc_pool_out", [B, D], F16)
    t["cc_attn_in"] = nc.dram_tensor("cc_attn_in", [B, D], F16)
    t["cc_attn_out"] = nc.dram_tensor("cc_attn_out", [B, D], F16,
                                      addr_space="Shared")
    t["cc_z_in"] = nc.dram_tensor("cc_z_in", [B, HID], F32)
    t["cc_z_out"] = nc.dram_tensor("cc_z_out", [B, HID], F32,
                                   addr_space="Shared")

    with tile.TileContext(nc) as tc:
        import contextlib
        with contextlib.ExitStack() as ctx:
            _build(nc, tc, t, ctx)
    nc.finalize()
    return nc


def _build(nc, tc, t, ctx):
    GROUPS = [list(range(NC))]

    singles = ctx.enter_context(tc.tile_pool(name="singles", bufs=1))
    stp = ctx.enter_context(tc.tile_pool(name="stp", bufs=6))    # stream ring
    vsc = ctx.enter_context(tc.tile_pool(name="vsc", bufs=8))    # vec scratch
    psb = ctx.enter_context(tc.tile_pool(name="psb", bufs=2, space="PSUM"))

    # ---- small persistent tiles -------------------------------------------
    ident = singles.tile([P, P], F32)
    make_identity(nc, ident)
    ident16 = singles.tile([P, P], F16)
    nc.vector.tensor_copy(out=ident16[:], in_=ident[:])
    ones8 = singles.tile([1, 16], F16)
    nc.vector.memset(ones8[:], 1.0)
    sh1_i = singles.tile([P, 1], I32)
    nc.vector.memset(sh1_i[:], 1)
    magic_i = singles.tile([P, 1], I32)
    nc.vector.memset(magic_i[:], 0x5F3759DF)

    # 16-row nat staging tiles for xbar transposes (rows 8..15 stay zero)
    pg16 = singles.tile([16, F1S], F16)       # p_nat, later g16
    nc.vector.memset(pg16[:], 0.0)
    ps16 = singles.tile([16, D], F16)         # poolh, later attn-AR out
    nc.vector.memset(ps16[:], 0.0)
    ctx16 = singles.tile([16, DH], F16)
    nc.vector.memset(ctx16[:], 0.0)
    x16 = singles.tile([16, HID], F16)
    nc.vector.memset(x16[:], 0.0)
    hb16 = singles.tile([16, 4 * HID], F16)
    nc.vector.memset(hb16[:], 0.0)
    pooled_nat = singles.tile([B, D], F16)    # A2A-in, later AR-in staging

    # transposed layouts
    pT = singles.tile([P, SC, 16], F16)
    poolhT = singles.tile([P, DC, 16], F16)
    ctxT = singles.tile([P, DH // P, 16], F16)
    aT = singles.tile([P, DC, 16], F16)
    gT = singles.tile([P, FC, 16], F16)
    xT = singles.tile([P, HC, 16], F16)
    xsT = singles.tile([P, HC, 16], F16)
    hbT = singles.tile([P, 4 * HID // P, 16], F16)

    # ---- constants (gpsimd SWDGE queue; all tiny) -------------------------
    u_sb = singles.tile([P, DC, 16], F8)
    nc.sync.dma_start(out=u_sb[:], in_=t["u_r"][:])
    sci_sb = singles.tile([H, 1], F32)
    nc.gpsimd.dma_start(out=sci_sb[:], in_=t["sc_inv"][:])
    bv_sb = singles.tile([1, DH], F16)
    nc.gpsimd.dma_start(out=bv_sb[:], in_=t["bv16"][:])
    bo_sb = singles.tile([1, D], F16)
    nc.gpsimd.dma_start(out=bo_sb[:], in_=t["bo16"][:])
    fix1_sb = singles.tile([2, F1S], F16)
    nc.gpsimd.dma_start(out=fix1_sb[:], in_=t["fix1"][:])
    rb_sb = singles.tile([1, HID], F16)
    nc.gpsimd.dma_start(out=rb_sb[:], in_=t["rb16"][:])
    fixb_sb = singles.tile([2, NBLK, 4 * HID], F16)
    nc.gpsimd.dma_start(out=fixb_sb[:], in_=t["fixb"][:])
    bb2_sb = singles.tile([1, NBLK, HID], F16)
    nc.gpsimd.dma_start(out=bb2_sb[:], in_=t["blk_b2_16"][:].rearrange("n f -> (n f)")[None, :])
    rc_sb = singles.tile([TD, HID], F16)
    nc.gpsimd.dma_start(out=rc_sb[:], in_=t["rin_cond8"][:])
    rna_sb = singles.tile([AD, HID], F16)
    nc.gpsimd.dma_start(out=rna_sb[:], in_=t["rin_na8"][:])
    naT_sb = singles.tile([AD, B], F16)
    nc.gpsimd.dma_start(out=naT_sb[:], in_=t["naT"][:])
    ow_sb = singles.tile([P, HC, AD], F16)
    nc.gpsimd.dma_start(out=ow_sb[:],
                        in_=t["out_w"][:].rearrange("(c p) a -> p c a", p=P))
    ob_bc = singles.tile([B, AD], F32)
    nc.gpsimd.dma_start(out=ob_bc[:], in_=_bcast(t["out_bc"][:], B))
    fw_sb = singles.tile([TD, 1], F32)
    nc.gpsimd.dma_start(out=fw_sb[:], in_=t["four_w2"][:])
    ph_sb = singles.tile([TD, 1], F32)
    nc.gpsimd.dma_start(out=ph_sb[:], in_=t["phase2"][:])
    tb32 = singles.tile([TD, B], F32)
    nc.gpsimd.dma_start(out=tb32[:], in_=_bcast(t["timeT"][:], TD))
    cw1_sb = singles.tile([TD, 2 * TD], F16)
    nc.gpsimd.dma_start(out=cw1_sb[:], in_=t["cond_w1"][:])
    cb1_sb = singles.tile([2 * TD, 1], F32)
    nc.gpsimd.dma_start(out=cb1_sb[:], in_=t["cond_b1c"][:])
    cw2_sb = singles.tile([2 * TD, TD], F16)
    nc.gpsimd.dma_start(out=cw2_sb[:], in_=t["cond_w2"][:])
    cb2_sb = singles.tile([TD, 1], F32)
    nc.gpsimd.dma_start(out=cb2_sb[:], in_=t["cond_b2c"][:])
    bw1_sb = singles.tile([P, NBLK, HC, 4 * HID], F16)
    bw2_sb = singles.tile([P, NBLK, 4 * HID // P, HID], F16)
    rp8_sb = singles.tile([P, DC, HID], F16)
    w2q_sb = singles.tile([P, FC, HID], F16)

    # ---- helpers ----------------------------------------------------------
    def tdma(dst_T, src16):
        """(16, c*128) f16 sbuf -> (128, c, 16) sbuf via xbar DMA transpose.
        dst[p, c, j] = src[j, c*128+p]; rows 8..15 of src are zero."""
        nc.scalar.dma_start(out=dst_T[:], in_=src16, transpose=True)

    def bias_mm(ps, bias_row, n_total, stop=True):
        nch = (n_total + 511) // 512
        for n in range(nch):
            w = min(512, n_total - n * 512)
            nc.tensor.matmul(ps[:, n * 512:n * 512 + w], ones8[:, :B],
                             bias_row[:, n * 512:n * 512 + w],
                             start=False, stop=(stop and n == nch - 1))

    def dummy_mm(wt_slice):
        """Tiny keep-alive matmul gated on a stream tile's DMA."""
        d_ps = psb.tile([8, 512], F32, tag="ps", name=f"dps_{dummy_mm.n}")
        dummy_mm.n += 1
        nc.tensor.matmul(d_ps[:8, :64], wt_slice[:, :8], wt_slice[:, :64],
                         start=True, stop=True)
    dummy_mm.n = 0

    def ln_stats(x_nat, npart, n, uid, newton=2):
        """Return (negmu_irr_row [2,16] f16 tile, r [npart,1] f32 AP).
        row0 = -mean, row1 = 1/r = sqrt(var+eps)."""
        nsub = max(1, n // 512)
        st = vsc.tile([npart, nsub, nc.vector.BN_STATS_DIM], F32, tag="v",
                      name=f"lnst_{uid}")
        xg = x_nat.rearrange("p (a b) -> p a b", a=nsub)
        for g in range(nsub):
            nc.vector.bn_stats(out=st[:, g, :], in_=xg[:, g, :])
        mv = vsc.tile([npart, nc.vector.BN_AGGR_DIM], F32, tag="v",
                      name=f"lnmv_{uid}")
        nc.vector.bn_aggr(out=mv[:], in_=st[:])
        ve = vsc.tile([npart, 4], F32, tag="v", name=f"lnve_{uid}")
        nc.vector.tensor_scalar_add(out=ve[:, 0:1], in0=mv[:, 1:2], scalar1=1e-5)
        yi = vsc.tile([npart, 1], I32, tag="v", name=f"lnyi_{uid}")
        nc.vector.tensor_tensor(out=yi[:], in0=ve[:, 0:1].bitcast(I32),
                                in1=sh1_i[:npart, :],
                                op=ALU.logical_shift_right)
        nc.vector.tensor_tensor(out=yi[:], in0=magic_i[:npart, :], in1=yi[:],
                                op=ALU.subtract)
        r = yi[:].bitcast(F32)
        tt = vsc.tile([npart, 1], F32, tag="v", name=f"lntt_{uid}")
        for _ in range(newton):
            nc.vector.tensor_mul(out=tt[:], in0=r, in1=r)
            nc.vector.tensor_mul(out=tt[:], in0=tt[:], in1=ve[:, 0:1])
            nc.vector.tensor_scalar(out=tt[:], in0=tt[:], scalar1=-0.5,
                                    scalar2=1.5, op0=ALU.mult, op1=ALU.add)
            nc.vector.tensor_mul(out=yi[:].bitcast(F32), in0=r, in1=tt[:])
        pk = vsc.tile([npart, 2], F16, tag="v", name=f"lnpk_{uid}")
        nc.vector.tensor_scalar_mul(out=pk[:, 0:1], in0=mv[:, 0:1], scalar1=-1.0)
        nc.vector.tensor_mul(out=ve[:, 1:2], in0=ve[:, 0:1], in1=r)
        nc.vector.tensor_copy(out=pk[:, 1:2], in_=ve[:, 1:2])
        tp = psb.tile([8, 512], F16, tag="ps", name=f"lntp_{uid}")
        nc.tensor.transpose(tp[:2, :npart], pk[:], ident16[:npart, :npart])
        row = vsc.tile([2, 16], F16, tag="v", name=f"lnrow_{uid}")
        nc.vector.tensor_copy(out=row[:, :npart], in_=tp[:2, :npart])
        return row, r

    # =======================================================================
    # PE warmup (~3us of cold matmuls opens the HAM clock gate)
    # =======================================================================
    ps_w = psb.tile([8, 2048], F32, tag="ps", name="ps_warm")
    for i in range(26):
        nc.tensor.matmul(ps_w[:, :P], ident16[:, :B], ident16[:],
                         start=(i == 0), stop=(i == 25))

    # =======================================================================
    # STEP 1: scoresT (8, 2048) = U.T @ llmT   fp8 DoubleRow (K=256 per mm)
    # =======================================================================
    ps_sc = psb.tile([8, 2048], F32, tag="ps", name="ps_sc")
    llmT_r = t["llmT"].rearrange("(a p) s -> p a s", p=P)
    for j in range(DC // 8):
        lt = stp.tile([P, 8, S], F8, tag="s", name=f"llmT_t{j}")
        nc.sync.dma_start(out=lt[:, :4, :], in_=llmT_r[:, 8 * j:8 * j + 4, :])
        nc.sync.dma_start(out=lt[:, 4:, :], in_=llmT_r[:, 8 * j + 4:8 * j + 8, :])
        for kk in range(4):
            k2 = 4 * j + kk          # DoubleRow pair index (of DC//2)
            for n in range(S // 512):
                nc.tensor.matmul(
                    ps_sc[:, n * 512:(n + 1) * 512],
                    u_sb[:, 8 * j + 2 * kk:8 * j + 2 * kk + 2, :8],
                    lt[:, 2 * kk:2 * kk + 2, n * 512:(n + 1) * 512],
                    start=(k2 == 0), stop=(k2 == DC // 2 - 1),
                    perf_mode=DR)

    # =======================================================================
    # STEP 2: softmax over S (shift-invariant; |scores| small, exp is safe)
    # =======================================================================
    den = singles.tile([H, 1], F32)
    nc.scalar.activation(out=pg16[:8, :], in_=ps_sc[:], func=AF.Exp,
                         scale=sci_sb[:], accum_out=den[:])
    nc.vector.reciprocal(out=den[:], in_=den[:])
    tdma(pT, pg16[:])

    # =======================================================================
    # STEP 3: pooled (8, 4096) = pT.T @ llm ; AllToAll (head <-> batch)
    # =======================================================================
    ps_pA = psb.tile([8, 2048], F32, tag="ps", name="ps_poolA")
    ps_pB = psb.tile([8, 2048], F32, tag="ps", name="ps_poolB")
    llm_r = t["llm"].rearrange("(a p) d -> p a d", p=P)
    for j in range(SC // 2):
        lt = stp.tile([P, 2, D], F16, tag="s", name=f"llm_t{j}")
        nc.sync.dma_start(out=lt[:, 0:1, :], in_=llm_r[:, 2 * j:2 * j + 1, :])
        nc.sync.dma_start(out=lt[:, 1:2, :], in_=llm_r[:, 2 * j + 1:2 * j + 2, :])
        for kk in range(2):
            s = 2 * j + kk
            for n in range(4):
                nc.tensor.matmul(ps_pA[:, n * 512:(n + 1) * 512],
                                 pT[:, s, :8],
                                 lt[:, kk, n * 512:(n + 1) * 512],
                                 start=(s == 0), stop=(s == SC - 1))
            for n in range(4):
                nc.tensor.matmul(ps_pB[:, n * 512:(n + 1) * 512],
                                 pT[:, s, :8],
                                 lt[:, kk, HD2 + n * 512:HD2 + (n + 1) * 512],
                                 start=(s == 0), stop=(s == SC - 1))
    # evict halves on two engines in parallel (8-lane ops are slow)
    nc.scalar.activation(out=pooled_nat[:, :HD2], in_=ps_pA[:],
                         func=AF.Identity, scale=den[:])
    nc.vector.tensor_scalar_mul(out=pooled_nat[:, HD2:], in0=ps_pB[:],
                                scalar1=den[:])
    nc.scalar.dma_start(out=t["cc_pool_in"][:], in_=pooled_nat[:])
    nc.gpsimd.collective_compute(
        "AllToAll", ALU.bypass, replica_groups=GROUPS,
        ins=[t["cc_pool_in"][:].opt()], outs=[t["cc_pool_out"][:].opt()])

    # ---- weight stream on the fat sync queue (starts after llm tiles) ----
    wv_tiles = []
    for g in range(2):
        wt = stp.tile([P, 16, DH], F16, tag="s", name=f"wv_g{g}")
        nc.sync.dma_start(out=wt[:, :8, :], in_=t["wv_r"][:, 16 * g:16 * g + 8, :])
        dummy_mm(wt[:, 0, :])
        nc.sync.dma_start(out=wt[:, 8:, :], in_=t["wv_r"][:, 16 * g + 8:16 * g + 16, :])
        dummy_mm(wt[:, 8, :])
        wv_tiles.append(wt)
    wo_tiles = []
    for g in range(2):
        wt = stp.tile([P, 2, D], F16, tag="s", name=f"wo_g{g}")
        nc.sync.dma_start(out=wt[:, 0:1, :], in_=t["wo_r"][:, 2 * g:2 * g + 1, :])
        dummy_mm(wt[:, 0, :])
        nc.sync.dma_start(out=wt[:, 1:2, :], in_=t["wo_r"][:, 2 * g + 1:2 * g + 2, :])
        dummy_mm(wt[:, 1, :])
        wo_tiles.append(wt)
    nc.sync.dma_start(out=rp8_sb[:], in_=t["rp8_r"][:])
    dummy_mm(rp8_sb[:, 0, :])
    nc.sync.dma_start(out=w2q_sb[:], in_=t["w2q_r"][:])
    dummy_mm(w2q_sb[:, 0, :])
    nc.sync.dma_start(out=bw1_sb[:], in_=t["bw1_r"][:])
    dummy_mm(bw1_sb[:, 0, 0, :])
    nc.sync.dma_start(out=bw2_sb[:], in_=t["bw2_r"][:])
    dummy_mm(bw2_sb[:, 0, 0, :])

    # w1 stream: 8 tiles of [P, 4, F1S]; dummies only where the gating DMA's
    # ring slot is freed by compute that precedes them in PE FIFO order.
    w1_tiles = []
    for g in range(8):
        wt = stp.tile([P, 4, F1S], F16, tag="s", name=f"w1_g{g}")
        nc.sync.dma_start(out=wt[:, :2, :], in_=t["w1_r"][:, 4 * g:4 * g + 2, :])
        nc.sync.dma_start(out=wt[:, 2:, :], in_=t["w1_r"][:, 4 * g + 2:4 * g + 4, :])
        if g < 2:   # slots of later tiles are freed by ctx/attn/mm1,
            # which sit after these dummies in PE FIFO order
            dummy_mm(wt[:, 0, :])
        w1_tiles.append(wt)

    # ---- cond path (independent; scheduled into the A2A gap) --------------
    fu = singles.tile([TD, B], F32)
    nc.vector.tensor_scalar_mul(out=fu[:], in0=tb32[:], scalar1=fw_sb[:])
    fi = singles.tile([TD, B], I32)
    nc.vector.tensor_copy(out=fi[:], in_=fu[:])
    fif = singles.tile([TD, B], F32)
    nc.vector.tensor_copy(out=fif[:], in_=fi[:])
    nc.vector.tensor_sub(out=fu[:], in0=fu[:], in1=fif[:])
    ffT = singles.tile([TD, B], F16)
    nc.scalar.activation(out=ffT[:], in_=fu[:], func=AF.Sin,
                         scale=TWO_PI, bias=ph_sb[:])
    ps_c1 = psb.tile([P, 8], F32, tag="ps", name="ps_c1")
    nc.tensor.matmul(ps_c1[:2 * TD, :B], cw1_sb[:], ffT[:], start=True,
                     stop=True)
    c1 = singles.tile([2 * TD, B], F16)
    nc.scalar.activation(out=c1[:], in_=ps_c1[:2 * TD, :B], func=AF.Silu,
                         bias=cb1_sb[:])
    ps_c2 = psb.tile([P, 8], F32, tag="ps", name="ps_c2")
    nc.tensor.matmul(ps_c2[:TD, :B], cw2_sb[:], c1[:], start=True, stop=True)
    condT = singles.tile([TD, B], F16)
    nc.scalar.activation(out=condT[:], in_=ps_c2[:TD, :B], func=AF.Identity,
                         bias=cb2_sb[:])

    # =======================================================================
    # STEP 4: A2A out -> poolhT ; ctx (8, 512) = poolh @ wv + bv
    # =======================================================================
    nc.scalar.dma_start(out=ps16[:8, :], in_=t["cc_pool_out"][:])
    tdma(poolhT, ps16[:])
    ps_cx = psb.tile([8, 2048], F32, tag="ps", name="ps_cx")
    for g in range(2):
        for j in range(16):
            k = 16 * g + j
            nc.tensor.matmul(ps_cx[:, :DH], poolhT[:, k, :8],
                             wv_tiles[g][:, j, :],
                             start=(k == 0), stop=False)
    bias_mm(ps_cx[:, :DH], bv_sb, DH)
    nc.scalar.activation(out=ctx16[:8, :], in_=ps_cx[:, :DH], func=AF.Identity)
    tdma(ctxT, ctx16[:])

    # =======================================================================
    # STEP 5: attn partial (8, 4096) = ctx @ wo + bo/8 ; AllReduce (f16).
    # Needed only for LN stats and the a@rp term of z.
    # =======================================================================
    ps_aA = psb.tile([8, 2048], F32, tag="ps", name="ps_attnA")
    ps_aB = psb.tile([8, 2048], F32, tag="ps", name="ps_attnB")
    for g in range(2):
        for kk in range(2):
            k = 2 * g + kk
            for n in range(4):
                nc.tensor.matmul(ps_aA[:, n * 512:(n + 1) * 512],
                                 ctxT[:, k, :8],
                                 wo_tiles[g][:, kk, n * 512:(n + 1) * 512],
                                 start=(k == 0), stop=False)
            for n in range(4):
                nc.tensor.matmul(ps_aB[:, n * 512:(n + 1) * 512],
                                 ctxT[:, k, :8],
                                 wo_tiles[g][:, kk, HD2 + n * 512:HD2 + (n + 1) * 512],
                                 start=(k == 0), stop=False)
    bias_mm(ps_aA, bo_sb[:, :HD2], HD2)
    bias_mm(ps_aB, bo_sb[:, HD2:], HD2)
    nc.scalar.activation(out=pooled_nat[:, :HD2], in_=ps_aA[:],
                         func=AF.Identity)
    nc.vector.tensor_copy(out=pooled_nat[:, HD2:], in_=ps_aB[:])
    nc.scalar.dma_start(out=t["cc_attn_in"][:], in_=pooled_nat[:])
    nc.gpsimd.collective_compute(
        "AllReduce", ALU.add, replica_groups=GROUPS,
        ins=[t["cc_attn_in"][:].opt()], outs=[t["cc_attn_out"][:].opt()])

    # =======================================================================
    # STEP 6: a -> aT; z a-term; mm1 with LN folded:
    #   h1 = r*(a@W1g) - r*mu*csum + b1'  -> g = Gelu(r * PSUM)
    # =======================================================================
    nc.scalar.dma_start(out=ps16[:8, :], in_=t["cc_attn_out"][:])
    tdma(aT, ps16[:])
    row1, r1 = ln_stats(ps16[:8, :], 8, D, "ln0")

    ps_z = psb.tile([8, 2048], F32, tag="ps", name="ps_z")
    for k in range(DC):
        nc.tensor.matmul(ps_z[:, :HID], aT[:, k, :8], rp8_sb[:, k, :],
                         start=(k == 0), stop=False)

    ps_h1 = psb.tile([8, 2048], F32, tag="ps", name="ps_h1")
    for g in range(8):
        for kk in range(4):
            k = 4 * g + kk
            for n in range(4):
                nc.tensor.matmul(ps_h1[:, n * 512:(n + 1) * 512],
                                 aT[:, k, :8],
                                 w1_tiles[g][:, kk, n * 512:(n + 1) * 512],
                                 start=(k == 0), stop=False)
    for n in range(4):   # rank-1 fixups: (-mu) x csum + (1/r) x b1'
        nc.tensor.matmul(ps_h1[:, n * 512:(n + 1) * 512], row1[:, :8],
                         fix1_sb[:, n * 512:(n + 1) * 512],
                         start=False, stop=(n == 3))
    nc.scalar.activation(out=pg16[:8, :], in_=ps_h1[:], func=AF.Gelu,
                         scale=r1)
    tdma(gT, pg16[:])

    # =======================================================================
    # STEP 7: z = a@rp/8 + g@W2Q + cond@rc/8 + na@rna/8 + rb ; AllReduce
    # =======================================================================
    for k in range(FC):
        nc.tensor.matmul(ps_z[:, :HID], gT[:, k, :8], w2q_sb[:, k, :],
                         start=False, stop=False)
    nc.tensor.matmul(ps_z[:, :HID], condT[:], rc_sb[:], start=False, stop=False)
    nc.tensor.matmul(ps_z[:, :HID], naT_sb[:], rna_sb[:], start=False, stop=False)
    bias_mm(ps_z[:, :HID], rb_sb, HID)
    z_nat = singles.tile([B, HID], F32)
    nc.vector.tensor_copy(out=z_nat[:], in_=ps_z[:, :HID])
    nc.scalar.dma_start(out=t["cc_z_in"][:], in_=z_nat[:])
    nc.gpsimd.collective_compute(
        "AllReduce", ALU.add, replica_groups=GROUPS,
        ins=[t["cc_z_in"][:].opt()], outs=[t["cc_z_out"][:].opt()])

    # =======================================================================
    # STEP 8: diffusion tail (replicated; LN folded via the same fixup)
    # =======================================================================
    x_nat = singles.tile([B, HID], F32)
    nc.scalar.dma_start(out=x_nat[:], in_=t["cc_z_out"][:])
    nc.vector.tensor_copy(out=x16[:8, :], in_=x_nat[:])
    tdma(xT, x16[:])

    for i in range(NBLK):
        rowb, rb_ = ln_stats(x_nat[:], 8, HID, f"lnb{i}", newton=2)
        ps_bh = psb.tile([8, 2048], F32, tag="ps", name=f"ps_bh_{i}")
        for k in range(HC):
            for n in range(2):
                nc.tensor.matmul(ps_bh[:, n * 512:(n + 1) * 512],
                                 xT[:, k, :8],
                                 bw1_sb[:, i, k, n * 512:(n + 1) * 512],
                                 start=(k == 0), stop=False)
        for n in range(2):
            nc.tensor.matmul(ps_bh[:, n * 512:(n + 1) * 512], rowb[:, :8],
                             fixb_sb[:, i, n * 512:(n + 1) * 512],
                             start=False, stop=(n == 1))
        nc.scalar.activation(out=hb16[:8, :], in_=ps_bh[:, :4 * HID],
                             func=AF.Silu, scale=rb_)
        tdma(hbT, hb16[:])

        ps_bo = psb.tile([8, 2048], F32, tag="ps", name=f"ps_bo_{i}")
        for k in range(4 * HID // P):
            nc.tensor.matmul(ps_bo[:, :HID], hbT[:, k, :8],
                             bw2_sb[:, i, k, :],
                             start=(k == 0), stop=False)
        bias_mm(ps_bo[:, :HID], bb2_sb[:, i, :], HID)
        nc.vector.tensor_add(out=x_nat[:], in0=x_nat[:], in1=ps_bo[:, :HID])
        if i < NBLK - 1:
            nc.vector.tensor_copy(out=x16[:8, :], in_=x_nat[:])
            tdma(xT, x16[:])

    # ---- final: res (8, 7) = swish(x) @ out_w + out_b
    nc.scalar.activation(out=x16[:8, :], in_=x_nat[:], func=AF.Silu)
    tdma(xsT, x16[:])
    ps_o = psb.tile([8, 2048], F32, tag="ps", name="ps_o")
    for k in range(HC):
        nc.tensor.matmul(ps_o[:8, :AD], xsT[:, k, :8], ow_sb[:, k, :],
                         start=(k == 0), stop=(k == HC - 1))
    out_sb = singles.tile([B, AD], F32)
    nc.vector.tensor_add(out=out_sb[:], in0=ps_o[:8, :AD], in1=ob_bc[:])
    nc.scalar.dma_start(out=t["res"][:], in_=out_sb[:])


_CACHED_NC = None


def _get_nc():
    global _CACHED_NC
    if _CACHED_NC is None:
        _CACHED_NC = build_program()
    return _CACHED_NC


def _prep_in_maps(inputs):
    f32 = np.float32
    f16 = np.float16
    llm_full = np.ascontiguousarray(np.asarray(inputs["llm_output"], dtype=f32))
    wq = np.asarray(inputs["wq"], f32); wk = np.asarray(inputs["wk"], f32)
    wv = np.asarray(inputs["wv"], f32); wo = np.asarray(inputs["wo"], f32)
    bq = np.asarray(inputs["bq"], f32); bv = np.asarray(inputs["bv"], f32)
    bo = np.asarray(inputs["bo"], f32)
    w1 = np.asarray(inputs["mlp_w1"], f32); b1 = np.asarray(inputs["mlp_b1"], f32)
    w2 = np.asarray(inputs["mlp_w2"], f32); b2 = np.asarray(inputs["mlp_b2"], f32)
    ln_g = np.asarray(inputs["ln_g"], f32); ln_b = np.asarray(inputs["ln_b"], f32)
    rin_w = np.asarray(inputs["rin_w"], f32)
    probe = np.asarray(inputs["probe"], f32).reshape(D)

    q = probe @ wq + bq
    U = np.empty((D, H), f32)
    for h in range(H):
        hs = slice(h * DH, (h + 1) * DH)
        U[:, h] = wk[:, hs] @ q[hs]
    U *= 1.0 / np.sqrt(DH)
    f8 = mybir.dt.np(F8)
    u_scale = float(2.0 ** np.floor(np.log2(64.0 / max(np.abs(U).max(), 1e-30))))
    u_pad = np.zeros((DC, P, 16), f32)
    u_pad[:, :, :H] = (U * u_scale).reshape(DC, P, H)

    # LN affine fold: LN(x)*g+b @ W == LN(x) @ (g*W) + b@W
    w1g = w1 * ln_g[:, None]
    b1_fold = b1 + ln_b @ w1                  # (4*D,)

    rp = rin_w[TD:TD + D]                     # (D, HID)

    blk_g = np.asarray(inputs["blk_ln_g"], f32)
    blk_b = np.asarray(inputs["blk_ln_b"], f32)
    blk_w1 = np.asarray(inputs["blk_w1"], f32)
    blk_w2 = np.asarray(inputs["blk_w2"], f32)
    blk_b1 = np.asarray(inputs["blk_b1"], f32)
    bw1g = blk_w1 * blk_g[:, :, None]
    bb1_fold = blk_b1 + np.einsum("nh,nhf->nf", blk_b, blk_w1)
    fixb = np.stack([bw1g.sum(axis=1), bb1_fold])     # (2, NBLK, 4*HID)

    shared = {
        "u_r": np.ascontiguousarray(u_pad.transpose(1, 0, 2)).astype(f8),
        "sc_inv": np.full((H, 1), 1.0 / u_scale, f32),
        "bo16": (bo / NC).astype(f16).reshape(1, D),
        "four_w2": np.concatenate(
            [np.asarray(inputs["four_w"], f32).reshape(TD // 2, 1)] * 2),
        "phase2": np.concatenate(
            [np.full((TD // 2, 1), np.pi / 2, f32),
             np.zeros((TD // 2, 1), f32)]),
        "timeT": np.ascontiguousarray(np.asarray(inputs["time"], f32).T),
        "naT": np.ascontiguousarray(
            np.asarray(inputs["noisy_actions"], f32).T).astype(f16),
        "cond_w1": np.asarray(inputs["cond_w1"], f32).astype(f16),
        "cond_b1c": np.asarray(inputs["cond_b1"], f32).reshape(-1, 1),
        "cond_w2": np.asarray(inputs["cond_w2"], f32).astype(f16),
        "cond_b2c": np.asarray(inputs["cond_b2"], f32).reshape(-1, 1),
        "rin_cond8": (np.ascontiguousarray(rin_w[0:TD]) / NC).astype(f16),
        "rp8_r": np.ascontiguousarray(
            (rp / NC).reshape(DC, P, HID).transpose(1, 0, 2)).astype(f16),
        "rin_na8": (np.ascontiguousarray(rin_w[TD + D:]) / NC).astype(f16),
        "rb16": ((np.asarray(inputs["rin_b"], f32) + b2 @ rp)
                 / NC).astype(f16).reshape(1, HID),
        "bw1_r": np.ascontiguousarray(
            bw1g.reshape(NBLK, HC, P, 4 * HID).transpose(2, 0, 1, 3)
        ).astype(f16),
        "fixb": fixb.astype(f16),
        "bw2_r": np.ascontiguousarray(
            blk_w2.reshape(NBLK, 4 * HID // P, P, HID).transpose(2, 0, 1, 3)
        ).astype(f16),
        "blk_b2_16": np.asarray(inputs["blk_b2"], f32).astype(f16),
        "out_w": np.asarray(inputs["out_w"], f32).astype(f16),
        "out_bc": np.asarray(inputs["out_b"], f32).reshape(1, AD),
    }

    in_maps = []
    for i in range(NC):
        hb = slice(i * DH, (i + 1) * DH)
        fb = slice(i * F1S, (i + 1) * F1S)
        m = dict(shared)
        m["llm"] = llm_full[i].astype(f16)
        m["llmT"] = np.ascontiguousarray(llm_full[i].T).astype(f8)
        m["wv_r"] = np.ascontiguousarray(
            wv[:, hb].reshape(DC, P, DH).transpose(1, 0, 2)).astype(f16)
        m["bv16"] = np.ascontiguousarray(bv[hb]).astype(f16).reshape(1, DH)
        m["wo_r"] = np.ascontiguousarray(
            wo[hb, :].reshape(4, P, D).transpose(1, 0, 2)).astype(f16)
        w1s = w1g[:, fb]
        m["w1_r"] = np.ascontiguousarray(
            w1s.reshape(DC, P, F1S).transpose(1, 0, 2)).astype(f16)
        m["fix1"] = np.stack([w1s.sum(axis=0), b1_fold[fb]]).astype(f16)
        m["w2q_r"] = np.ascontiguousarray(
            (w2[fb, :] @ rp).reshape(FC, P, HID).transpose(1, 0, 2)
        ).astype(f16)
        in_maps.append(m)
    return in_maps


def kernel(**inputs):
    nc = _get_nc()
    in_maps = _prep_in_maps(inputs)
    r = run_bass_kernel_spmd(nc, in_maps, core_ids=list(range(NC)))
    return np.ascontiguousarray(r.results[0]["res"]).astype(np.float32)


def run_traced(**inputs):
    nc = _get_nc()
    in_maps = _prep_in_maps(inputs)
    r = run_bass_kernel_spmd(nc, in_maps, core_ids=list(range(NC)), trace=True)
    return np.ascontiguousarray(r.results[0]["res"]).astype(np.float32), r
